# revision 46
# baseline (speedup 1.0000x reference)
"""EnhancedPolarAttention Trainium2 Bass kernel.

Full inputs in, full output out. Head-parallel across 8 NeuronCores
(1 head per core). See bottom of file for the host-side kernel() entry.

Math: scores = (q.k)/sqrt(hd) * r_w[j] * cos(theta_i - theta_j)
   with cos(a-b) = cos a cos b + sin a sin b, this is exactly
   q'_i . k'_j  with
   q' = [q * cos(theta_i), q * sin(theta_i)] / sqrt(hd)   (64-dim)
   k' = [k * r_w cos(theta_j), k * r_w sin(theta_j)]      (64-dim)
so the polar modulation folds into the QK^T matmul (contraction 64).

Scores are tiny (|s| < ~0.5), so softmax needs no max subtraction:
exp on ScalarE directly, and the denominator Z = sum_j exp(s_ij) is
obtained for free by augmenting v with a ones column inside the
attn @ v matmul (out row 32 = Z).

Everything is computed in a transposed (feature-major) layout so no
on-device transposes are needed at all:
  xT [C=128, N=4096]          (host pre-transposed, replicated)
  qk' [128, 4096]             rows 0-63 = q'T, rows 64-127 = k'T
  scoresT tile [128 keys, 512 queries] = k'chunk.T @ q'T  (PSUM)
  acc [33, 512] += v_aug_chunk.T @ exp(scoresT chunk)     (PSUM)
  outT [32, 512] = acc[0:32] * broadcast(1/acc[32])
  final [128 tok, 256] = outT_slice.T @ Wf_h  (+ bf/8)
Host sums the 8 per-head partial projections.
"""

import numpy as np

# ---- problem constants (hardcoded per contract) ----
B, HI, WI, C = 1, 64, 64, 128
N = HI * WI            # 4096
KEY_DIM = 256
NH = 8                 # heads
HD = KEY_DIM // NH     # 32
NCORES = 8
QC = 512               # query chunk = one PSUM bank of f32
NQC = N // QC          # 8
KC = 128               # key chunk = partition dim
NKC = N // KC          # 32
VW = HD + 1            # v augmented with ones column -> 33

_CACHE = {}


def _polar_constants():
    """Match reference._polar_constants in float32 numpy."""
    H, W = HI, WI
    y, x = np.meshgrid(np.arange(H, dtype=np.float32),
                       np.arange(W, dtype=np.float32))
    x = x.reshape(-1)
    y = y.reshape(-1)
    r = np.sqrt(np.square(x - W / 2) + np.square(y - H / 2)).astype(np.float32) + np.float32(1e-6)
    theta = np.arctan2(y - H / 2, x - W / 2).astype(np.float32)
    log_r = (np.log(r) / np.log(r.max())).astype(np.float32)
    theta = ((theta + 2 * np.pi) % (2 * np.pi)).astype(np.float32)
    r_weight = (1.0 / (log_r + 1.0)).astype(np.float32)
    return r_weight, theta


def _build_nc(debug_taps=False):
    import concourse.mybir as mybir
    import concourse.tile as tile
    from concourse import bacc

    F32 = mybir.dt.float32
    BF16 = mybir.dt.float16  # fp16: same PE speed as bf16, 8x the mantissa
    EXP = mybir.ActivationFunctionType.Exp
    ADD = mybir.AluOpType.add
    MULT = mybir.AluOpType.mult

    nc = bacc.Bacc("TRN2", target_bir_lowering=False)
    dbg = {}
    if debug_taps:
        dbg["qp"] = nc.dram_tensor("dbg_qp", [64, N], F32, kind="ExternalOutput")
        dbg["kp"] = nc.dram_tensor("dbg_kp", [64, N], F32, kind="ExternalOutput")
        dbg["v"] = nc.dram_tensor("dbg_v", [128, NKC * VW], F32, kind="ExternalOutput")
        dbg["ex0"] = nc.dram_tensor("dbg_ex0", [128, 3 * QC], F32, kind="ExternalOutput")
        dbg["acc0"] = nc.dram_tensor("dbg_acc0", [VW, QC], F32, kind="ExternalOutput")
        dbg["recip0"] = nc.dram_tensor("dbg_recip0", [1, QC], F32, kind="ExternalOutput")
        dbg["rb0"] = nc.dram_tensor("dbg_rb0", [HD, QC], F32, kind="ExternalOutput")
        dbg["outT0"] = nc.dram_tensor("dbg_outT0", [HD, QC], F32, kind="ExternalOutput")

    xT_d = nc.dram_tensor("xT", [C, N], BF16, kind="ExternalInput")
    mcq_d = nc.dram_tensor("mcq", [128, N], F32, kind="ExternalInput")
    mck_d = nc.dram_tensor("mck", [128, N], F32, kind="ExternalInput")
    wqq_d = nc.dram_tensor("wqq", [C, 128], BF16, kind="ExternalInput")
    wkk_d = nc.dram_tensor("wkk", [C, 128], BF16, kind="ExternalInput")
    wv_d = nc.dram_tensor("wv", [C, HD], BF16, kind="ExternalInput")
    wf_d = nc.dram_tensor("wf", [HD, KEY_DIM], BF16, kind="ExternalInput")
    out_d = nc.dram_tensor("out", [N, KEY_DIM], F32, kind="ExternalOutput")
    z_d = nc.dram_tensor("z", [NQC, QC], F32, kind="ExternalOutput")

    with tile.TileContext(nc) as tc, \
         tc.tile_pool(name="singles", bufs=1) as singles, \
         tc.tile_pool(name="work", bufs=2) as work, \
         tc.tile_pool(name="psum", bufs=2, space="PSUM") as psum:

        # ---- persistent SBUF ----
        xT_sb = singles.tile([C, N], BF16)        # fp16 x (cast by DMA)
        mcq_sb = singles.tile([128, N], F32)
        mck_sb = singles.tile([128, N], F32)
        # q'/k' duplicated into both partition halves (rows 64-127 = rows
        # 0-63) so score matmuls for chunk pairs run CONCURRENTLY in the PE
        # array via disjoint row-groups (K=64 uses half the array).
        qp_sb = singles.tile([128, N], BF16)      # q'T x2
        kp_sb = singles.tile([128, N], BF16)      # k'T x2
        v_sb = singles.tile([128, NKC * VW], BF16)  # 32 chunks of [128, 33]
        wqq_sb = singles.tile([C, 128], BF16)
        wkk_sb = singles.tile([C, 128], BF16)
        wv_sb = singles.tile([C, HD], BF16)
        wf_sb = singles.tile([HD, KEY_DIM], BF16)

        nc.vector.memset(v_sb, 1.0)   # every 33rd column stays 1.0

        # ---- input DMAs: weights first (phase-A matmuls need them),
        # then x/constants in chunks so compute starts early ----
        nc.sync.dma_start(out=wqq_sb, in_=wqq_d[:, :])
        nc.sync.dma_start(out=wkk_sb, in_=wkk_d[:, :])
        nc.sync.dma_start(out=wv_sb, in_=wv_d[:, :])
        nc.sync.dma_start(out=wf_sb, in_=wf_d[:, :])
        for half in range(2):
            s = slice(half * (N // 2), (half + 1) * (N // 2))
            nc.sync.dma_start(out=xT_sb[:, s], in_=xT_d[:, s])
        for half in range(2):
            s = slice(half * (N // 2), (half + 1) * (N // 2))
            nc.sync.dma_start(out=mcq_sb[:, s], in_=mcq_d[:, s])
            nc.sync.dma_start(out=mck_sb[:, s], in_=mck_d[:, s])

        # ---- phase A: q'/k' projection with polar modulation fused ----
        # ps_q = Wqqqq.T @ xT -> [128 feat(q,q,q,q), 512 tok] (weight columns
        # duplicated host-side so rows 64-127 replicate rows 0-63 for free);
        # q' = ps_q * mcq.  mcq rows: cos/sqrt(hd), sin/sqrt(hd), repeated.
        # Only chunk 0 runs up front; chunks 1-7 and the v-projection are
        # interleaved into qc0's pair loop (qc0 needs qp chunk 0 and kp
        # chunks progressively), which cuts the ramp to first exp.
        def emit_proj_part(i, which):
            s = slice(i * QC, (i + 1) * QC)
            w_sb = wqq_sb if which == "q" else wkk_sb
            m_sb = mcq_sb if which == "q" else mck_sb
            p_sb = qp_sb if which == "q" else kp_sb
            ps = psum.tile([128, QC], F32, tag="s", name=f"ps_{which}_{i}")
            nc.tensor.matmul(ps, w_sb, xT_sb[:, s], start=True, stop=True)
            nc.vector.tensor_mul(p_sb[:, s], ps, m_sb[:, s])

        def emit_proj_chunk(i):
            emit_proj_part(i, "q")
            emit_proj_part(i, "k")

        # v projection: 4 chunks share one PSUM slot and one strided DVE
        # copy into the 33-strided v_sb blocks.
        def emit_v_group(j4):
            ps_v = psum.tile([128, 4 * HD], F32, tag="s", name=f"ps_v_{j4}")
            for u in range(4):
                j = 4 * j4 + u
                nc.tensor.matmul(ps_v[:, u * HD:(u + 1) * HD],
                                 xT_sb[:, j * KC:(j + 1) * KC], wv_sb,
                                 start=True, stop=True, skip_group_check=True)
            v_view = v_sb[:, j4 * 4 * VW:(j4 * 4 + 4) * VW].rearrange(
                "p (j w) -> p j w", w=VW)[:, :, 0:HD]
            nc.vector.tensor_copy(
                v_view, ps_v[:, :].rearrange("p (j w) -> p j w", w=HD))

        emit_proj_chunk(0)
        emit_v_group(0)

        # ---- phase B: attention main loop ----
        # groups of 3 key-chunks share one 3-bank PSUM tile so exp runs as
        # one ACT instruction over [128, 1536]
        GROUPS = [3] * 10 + [2]
        if debug_taps:
            nc.gpsimd.dma_start(out=dbg["qp"][:, :], in_=qp_sb)
            nc.gpsimd.dma_start(out=dbg["kp"][:, :], in_=kp_sb)
            nc.gpsimd.dma_start(out=dbg["v"][:, :], in_=v_sb)
        # chunk k -> (group, slot) for the fixed GROUPS partition
        chunk_grp = []
        kb = 0
        for g, gs in enumerate(GROUPS):
            for t in range(gs):
                chunk_grp.append((g, t))
            kb += gs
        grp_base = []
        kb = 0
        for gs in GROUPS:
            grp_base.append(kb)
            kb += gs

        # Deferred per-qc tail (recip -> broadcast -> normalize -> final
        # projection -> DMA out). Emitted a few pairs INTO the next query
        # chunk so the in-order PE reaches the next chunk's score matmuls
        # (feeding ACT) before it queues behind the DVE normalize chain.
        def emit_norm_for(q, st):
            # No on-device softmax normalization: proj(out/Z) == proj(out)/Z
            # per token, so Z ships to the host (z_d) and the divide folds
            # into the host-side gather. Here: just cast outT to fp16.
            accs = st["accs"]
            outT = work.tile([HD, QC], BF16, tag="o", bufs=2,
                             name=f"outT_{q}")
            nc.vector.tensor_copy(outT, accs[0:HD, :])
            st["outT"] = outT
            st["os4"] = work.tile([128, 4, KEY_DIM], F32, tag="os", bufs=2,
                                  name=f"os4_{q}")

        def emit_proj_for(q, st, h2, dma_now=False):
            outT, os4 = st["outT"], st["os4"]
            pf = psum.tile([128, 2 * KEY_DIM], F32, tag="x", bufs=1,
                           name=f"pf_{q}_{h2}")
            for u in range(2):
                t = 2 * h2 + u
                nc.tensor.matmul(pf[:, u * KEY_DIM:(u + 1) * KEY_DIM],
                                 outT[:, t * 128:(t + 1) * 128], wf_sb,
                                 start=True, stop=True,
                                 skip_group_check=True)
            nc.vector.tensor_copy(os4[:, 2 * h2:2 * h2 + 2, :],
                                  pf[:, :].rearrange("p (u c) -> p u c",
                                                     c=KEY_DIM))
            if dma_now:
                out_view = out_d[:, :].rearrange(
                    "(qh t p) c -> qh p t c", t=2, p=128, qh=2 * NQC)
                nc.sync.dma_start(out=out_view[2 * q + h2],
                                  in_=os4[:, 2 * h2:2 * h2 + 2, :])

        def emit_out_dma_for(q, st):
            out_view = out_d[:, :].rearrange("(q t p) c -> q p t c", t=4, p=128)
            nc.sync.dma_start(out=out_view[q], in_=st["os4"])

        def emit_merge_for(q, acc, st):
            # Merge the two accumulators while copying out of PSUM (frees
            # the accumulator bank; the slow normalize chain runs on the
            # SBUF copy, deferred further into the next query chunk).
            accs = work.tile([VW, QC], F32, tag="accs", bufs=2,
                             name=f"accs_{q}")
            nc.vector.tensor_copy(accs, acc[0:VW, :])
            nc.vector.scalar_tensor_tensor(
                out=accs, in0=acc[64:64 + VW, :], scalar=1.0, in1=accs,
                op0=MULT, op1=ADD)
            if debug_taps and q == 0:
                nc.sync.dma_start(out=dbg["acc0"][:, :], in_=accs)
            nc.sync.dma_start(out=z_d[q:q + 1, :], in_=accs[HD:HD + 1, :])
            st["accs"] = accs

        # Work carried across the qc boundary, drained one item per score
        # pair so the in-order PE keeps feeding ACT with the next chunk's
        # scores instead of queueing behind the previous chunk's epilogue.
        deferred = []

        for q in range(NQC):
            qs = slice(q * QC, (q + 1) * QC)
            # Two accumulators (partitions 0-32 even chunks, 64-96 odd) so
            # attnv matmuls of a chunk pair run concurrently via disjoint
            # col-groups; merged after the loop.
            acc = psum.tile([97, QC], F32, tag="acc", bufs=1,
                            name=f"acc_{q}")

            # Scores are emitted in chunk PAIRS: even chunk uses array rows
            # 0-63, odd chunk rows 64-127 (duplicated q'/k' halves) -> the
            # two matmuls execute concurrently in the PE array. exp(g) is
            # emitted as soon as group g's last score lands; attnv(g) is
            # delayed by one group so the in-order PE never stalls on ACT.
            sc_tiles = {}
            emitted = [0] * len(GROUPS)
            attnv_q = []

            def emit_exp(g, q=q, sc_tiles=sc_tiles, attnv_q=attnv_q):
                gs = GROUPS[g]
                ex = work.tile([128, gs * QC], BF16, tag="e", bufs=3,
                               name=f"ex_{q}_{g}")
                nc.scalar.activation(ex, sc_tiles[g], EXP)
                if debug_taps and q == 0 and g == 0:
                    nc.gpsimd.dma_start(out=dbg["ex0"][:, :], in_=ex)
                attnv_q.append((g, ex))

            def emit_attnv(g, ex, acc=acc):
                gs = GROUPS[g]
                for t in range(gs):
                    k = grp_base[g] + t
                    odd = k % 2
                    nc.tensor.matmul(
                        acc[64:97, :] if odd else acc[0:33, :],
                        v_sb[:, k * VW:(k + 1) * VW],         # [128, 33]
                        ex[:, t * QC:(t + 1) * QC],           # [128, 512]
                        start=(k < 2), stop=(k >= NKC - 2),
                        tile_position=(0, 64) if odd else (0, 0),
                        skip_group_check=True)

            for p in range(NKC // 2):
                for k in (2 * p, 2 * p + 1):
                    g, slot = chunk_grp[k]
                    if g not in sc_tiles:
                        sc_tiles[g] = psum.tile(
                            [128, GROUPS[g] * QC], F32, tag="s", bufs=2,
                            name=f"sc_{q}_{g}")
                    half = 64 * (k % 2)
                    nc.tensor.matmul(
                        sc_tiles[g][:, slot * QC:(slot + 1) * QC],
                        kp_sb[half:half + 64, k * KC:(k + 1) * KC],
                        qp_sb[half:half + 64, qs],
                        start=True, stop=True)
                    emitted[g] += 1
                    if emitted[g] == GROUPS[g]:
                        emit_exp(g)
                        # lag attnv by one group behind exp
                        if len(attnv_q) >= 2:
                            emit_attnv(*attnv_q.pop(0))
                # previous-qc epilogue, one piece every other pair so each
                # item's producer is long done when the in-order PE gets
                # there (no mid-stream stalls in front of score matmuls)
                if deferred and p % 2 == 0:
                    deferred.pop(0)()
                if q == 0:
                    # rest of phase A, raced just ahead of its consumers;
                    # qp chunk i is only needed at qc i so those spread
                    # across later qcs (emitted one qc ahead)
                    if p % 2 == 0 and p // 2 + 1 < NQC:
                        emit_proj_part(p // 2 + 1, "k")
                    if p % 2 == 1 and (p + 1) // 2 < NKC // 4:
                        emit_v_group((p + 1) // 2)
                elif p == 9 and q + 1 < NQC:
                    emit_proj_part(q + 1, "q")
                if q == 0 and p == 15:
                    emit_proj_part(1, "q")

            # carry this qc's epilogue into the next qc's pair loop
            def flush(q=q, acc=acc, attnv_q=attnv_q, emit_attnv=emit_attnv):
                st = {}
                steps = []
                for g, ex in attnv_q:
                    steps.append(lambda g=g, ex=ex: emit_attnv(g, ex))
                steps.append(lambda: emit_merge_for(q, acc, st))
                steps.append(lambda: emit_norm_for(q, st))
                steps.append(lambda: emit_proj_for(q, st, 0))
                steps.append(lambda: emit_proj_for(q, st, 1))
                steps.append(lambda: emit_out_dma_for(q, st))
                return steps

            if q == NQC - 1:
                # final epilogue inline, DMA halves overlapped with proj
                st = {}
                for g, ex in attnv_q:
                    emit_attnv(g, ex)
                emit_merge_for(q, acc, st)
                emit_norm_for(q, st)
                emit_proj_for(q, st, 0, dma_now=True)
                emit_proj_for(q, st, 1, dma_now=True)
                deferred = []
            else:
                deferred = flush()

    nc.compile()
    return nc

    nc.compile()
    return nc


def _prepare_inputs(x, Wp, bp, Wf, bf):
    """Build per-core input maps (head h -> core h)."""
    x = np.ascontiguousarray(x, dtype=np.float32)
    Wp = np.ascontiguousarray(Wp, dtype=np.float32)
    bp = np.ascontiguousarray(bp, dtype=np.float32)
    Wf = np.ascontiguousarray(Wf, dtype=np.float32)
    bf = np.ascontiguousarray(bf, dtype=np.float32)

    r_w, theta = _polar_constants()
    inv_sqrt_hd = np.float32(1.0 / np.sqrt(np.float32(HD)))
    cos_t = np.cos(theta).astype(np.float32)
    sin_t = np.sin(theta).astype(np.float32)

    mcq = np.empty((128, N), dtype=np.float32)
    mcq[0:32, :] = cos_t * inv_sqrt_hd
    mcq[32:64, :] = sin_t * inv_sqrt_hd
    mcq[64:128, :] = mcq[0:64, :]
    mck = np.empty((128, N), dtype=np.float32)
    mck[0:32, :] = r_w * cos_t
    mck[32:64, :] = r_w * sin_t
    mck[64:128, :] = mck[0:64, :]

    xT = np.ascontiguousarray(x.reshape(N, C).T).astype(np.float16)  # [C, N]

    # NOTE: q/k biases (bp[0:512]) are NOT applied on device; they are zero
    # by the problem spec (fill=zeros). The v bias folds exactly into a
    # host-side output bias since softmax rows sum to 1:
    #   p @ (v + bv) @ Wf_h = p @ v @ Wf_h + bv @ Wf_h
    assert np.max(np.abs(bp[:2 * KEY_DIM])) == 0.0, "nonzero q/k bias unsupported"
    bv_full = bp[2 * KEY_DIM:3 * KEY_DIM]
    host_bias = (bf + bv_full @ Wf).astype(np.float32)  # [256]

    in_maps = []
    for h in range(NCORES):
        qs = slice(32 * h, 32 * h + 32)
        Wq = Wp[:, 0 * KEY_DIM:1 * KEY_DIM][:, qs]
        Wk = Wp[:, 1 * KEY_DIM:2 * KEY_DIM][:, qs]
        Wv = Wp[:, 2 * KEY_DIM:3 * KEY_DIM][:, qs]
        wqq = np.ascontiguousarray(
            np.concatenate([Wq, Wq, Wq, Wq], axis=1)).astype(np.float16)
        wkk = np.ascontiguousarray(
            np.concatenate([Wk, Wk, Wk, Wk], axis=1)).astype(np.float16)
        wf_h = np.ascontiguousarray(Wf[qs, :]).astype(np.float16)  # [32, 256]
        in_maps.append({
            "xT": xT, "mcq": mcq, "mck": mck,
            "wqq": wqq, "wkk": wkk,
            "wv": np.ascontiguousarray(Wv).astype(np.float16),
            "wf": wf_h,
        })
    return in_maps, host_bias


def kernel(x, Wp, bp, Wf, bf):
    from concourse.bass_utils import run_bass_kernel_spmd

    if "nc" not in _CACHE:
        _CACHE["nc"] = _build_nc()
    nc = _CACHE["nc"]

    in_maps, host_bias = _prepare_inputs(x, Wp, bp, Wf, bf)
    res = run_bass_kernel_spmd(nc, in_maps, core_ids=list(range(NCORES)))
    out = _combine_outputs(res.results)
    out = out + host_bias[None, :]
    return out.reshape(B, HI, WI, KEY_DIM).astype(np.float32)


def _combine_outputs(results):
    """Sum per-head partials, folding in the softmax denominators."""
    out = np.zeros((N, KEY_DIM), dtype=np.float32)
    for r in results:
        z = np.asarray(r["z"], dtype=np.float32).reshape(N, 1)
        out += np.asarray(r["out"], dtype=np.float32) / z
    return out


# revision 47
# speedup vs baseline: 1.0179x; 1.0179x over previous
"""EnhancedPolarAttention Trainium2 Bass kernel.

Full inputs in, full output out. Head-parallel across 8 NeuronCores
(1 head per core). See bottom of file for the host-side kernel() entry.

Math: scores = (q.k)/sqrt(hd) * r_w[j] * cos(theta_i - theta_j)
   with cos(a-b) = cos a cos b + sin a sin b, this is exactly
   q'_i . k'_j  with
   q' = [q * cos(theta_i), q * sin(theta_i)] / sqrt(hd)   (64-dim)
   k' = [k * r_w cos(theta_j), k * r_w sin(theta_j)]      (64-dim)
so the polar modulation folds exactly into the QK^T matmul
(contraction 64) - no N x N elementwise modulation is ever formed.

Scores are tiny (|s| <= ~0.32 for these inputs), so softmax needs no
max subtraction: exp on ScalarE directly (batched [128,1536] PSUM
reads), and the denominator Z = sum_j exp(s_ij) falls out of the
attn @ v matmul for free by augmenting v with a ones column (row 32
of the accumulator). Normalization by 1/Z commutes with the final
per-head projection, so it is folded into the host-side gather
(z rows are shipped out with the partials).

Everything is computed in a transposed (feature-major) layout so no
on-device transposes are needed at all. All big matmuls run in fp16
(f32 runs LOW_HIGH double-pumped on the PE). q'/k' are duplicated
into both partition halves so score matmuls for chunk pairs execute
CONCURRENTLY in the PE array (disjoint row-groups, K=64 each);
attn-v matmuls pair via disjoint col-groups (M=33 each) into two
accumulator regions merged at the end. exp(group) is emitted as soon
as its scores land; attn-v lags one group; each query-chunk epilogue
(merge/cast/projection/DMA) is deferred piecewise into the next
chunk's pair loop so the in-order PE never stalls in front of score
matmuls. Per-core work: 2 x 256 matmuls [128x512] + 88 exp
instructions (~133us ScalarE busy, the bottleneck).
"""

import numpy as np

# ---- problem constants (hardcoded per contract) ----
B, HI, WI, C = 1, 64, 64, 128
N = HI * WI            # 4096
KEY_DIM = 256
NH = 8                 # heads
HD = KEY_DIM // NH     # 32
NCORES = 8
QC = 512               # query chunk = one PSUM bank of f32
NQC = N // QC          # 8
KC = 128               # key chunk = partition dim
NKC = N // KC          # 32
VW = HD + 1            # v augmented with ones column -> 33

_CACHE = {}


def _polar_constants():
    """Match reference._polar_constants in float32 numpy."""
    H, W = HI, WI
    y, x = np.meshgrid(np.arange(H, dtype=np.float32),
                       np.arange(W, dtype=np.float32))
    x = x.reshape(-1)
    y = y.reshape(-1)
    r = np.sqrt(np.square(x - W / 2) + np.square(y - H / 2)).astype(np.float32) + np.float32(1e-6)
    theta = np.arctan2(y - H / 2, x - W / 2).astype(np.float32)
    log_r = (np.log(r) / np.log(r.max())).astype(np.float32)
    theta = ((theta + 2 * np.pi) % (2 * np.pi)).astype(np.float32)
    r_weight = (1.0 / (log_r + 1.0)).astype(np.float32)
    return r_weight, theta


def _build_nc(debug_taps=False):
    import concourse.mybir as mybir
    import concourse.tile as tile
    from concourse import bacc

    F32 = mybir.dt.float32
    BF16 = mybir.dt.float16  # fp16: same PE speed as bf16, 8x the mantissa
    EXP = mybir.ActivationFunctionType.Exp
    ADD = mybir.AluOpType.add
    MULT = mybir.AluOpType.mult

    nc = bacc.Bacc("TRN2", target_bir_lowering=False)
    dbg = {}
    if debug_taps:
        dbg["qp"] = nc.dram_tensor("dbg_qp", [64, N], F32, kind="ExternalOutput")
        dbg["kp"] = nc.dram_tensor("dbg_kp", [64, N], F32, kind="ExternalOutput")
        dbg["v"] = nc.dram_tensor("dbg_v", [128, NKC * VW], F32, kind="ExternalOutput")
        dbg["ex0"] = nc.dram_tensor("dbg_ex0", [128, 3 * QC], F32, kind="ExternalOutput")
        dbg["acc0"] = nc.dram_tensor("dbg_acc0", [VW, QC], F32, kind="ExternalOutput")
        dbg["recip0"] = nc.dram_tensor("dbg_recip0", [1, QC], F32, kind="ExternalOutput")
        dbg["rb0"] = nc.dram_tensor("dbg_rb0", [HD, QC], F32, kind="ExternalOutput")
        dbg["outT0"] = nc.dram_tensor("dbg_outT0", [HD, QC], F32, kind="ExternalOutput")

    xT_d = nc.dram_tensor("xT", [C, N], BF16, kind="ExternalInput")
    mcq_d = nc.dram_tensor("mcq", [128, N], F32, kind="ExternalInput")
    mck_d = nc.dram_tensor("mck", [128, N], F32, kind="ExternalInput")
    wqq_d = nc.dram_tensor("wqq", [C, 128], BF16, kind="ExternalInput")
    wkk_d = nc.dram_tensor("wkk", [C, 128], BF16, kind="ExternalInput")
    wv_d = nc.dram_tensor("wv", [C, HD], BF16, kind="ExternalInput")
    wf_d = nc.dram_tensor("wf", [HD, KEY_DIM], BF16, kind="ExternalInput")
    out_d = nc.dram_tensor("out", [N, KEY_DIM], F32, kind="ExternalOutput")
    z_d = nc.dram_tensor("z", [NQC, QC], F32, kind="ExternalOutput")

    with tile.TileContext(nc) as tc, \
         tc.tile_pool(name="singles", bufs=1) as singles, \
         tc.tile_pool(name="work", bufs=2) as work, \
         tc.tile_pool(name="psum", bufs=2, space="PSUM") as psum:

        # ---- persistent SBUF ----
        xT_sb = singles.tile([C, N], BF16)        # fp16 x (cast by DMA)
        mcq_sb = singles.tile([128, N], F32)
        mck_sb = singles.tile([128, N], F32)
        # q'/k' duplicated into both partition halves (rows 64-127 = rows
        # 0-63) so score matmuls for chunk pairs run CONCURRENTLY in the PE
        # array via disjoint row-groups (K=64 uses half the array).
        qp_sb = singles.tile([128, N], BF16)      # q'T x2
        kp_sb = singles.tile([128, N], BF16)      # k'T x2
        v_sb = singles.tile([128, NKC * VW], BF16)  # 32 chunks of [128, 33]
        wqq_sb = singles.tile([C, 128], BF16)
        wkk_sb = singles.tile([C, 128], BF16)
        wv_sb = singles.tile([C, HD], BF16)
        wf_sb = singles.tile([HD, KEY_DIM], BF16)

        nc.vector.memset(v_sb, 1.0)   # every 33rd column stays 1.0

        # ---- input DMAs: weights first (phase-A matmuls need them),
        # then x/constants in chunks so compute starts early ----
        nc.sync.dma_start(out=wqq_sb, in_=wqq_d[:, :])
        nc.sync.dma_start(out=wkk_sb, in_=wkk_d[:, :])
        nc.sync.dma_start(out=wv_sb, in_=wv_d[:, :])
        nc.sync.dma_start(out=wf_sb, in_=wf_d[:, :])
        for half in range(2):
            s = slice(half * (N // 2), (half + 1) * (N // 2))
            nc.sync.dma_start(out=xT_sb[:, s], in_=xT_d[:, s])
        for half in range(2):
            s = slice(half * (N // 2), (half + 1) * (N // 2))
            nc.sync.dma_start(out=mcq_sb[:, s], in_=mcq_d[:, s])
            nc.sync.dma_start(out=mck_sb[:, s], in_=mck_d[:, s])

        # ---- phase A: q'/k' projection with polar modulation fused ----
        # ps_q = Wqqqq.T @ xT -> [128 feat(q,q,q,q), 512 tok] (weight columns
        # duplicated host-side so rows 64-127 replicate rows 0-63 for free);
        # q' = ps_q * mcq.  mcq rows: cos/sqrt(hd), sin/sqrt(hd), repeated.
        # Only chunk 0 runs up front; chunks 1-7 and the v-projection are
        # interleaved into qc0's pair loop (qc0 needs qp chunk 0 and kp
        # chunks progressively), which cuts the ramp to first exp.
        def emit_proj_part(i, which):
            s = slice(i * QC, (i + 1) * QC)
            w_sb = wqq_sb if which == "q" else wkk_sb
            m_sb = mcq_sb if which == "q" else mck_sb
            p_sb = qp_sb if which == "q" else kp_sb
            ps = psum.tile([128, QC], F32, tag="s", name=f"ps_{which}_{i}")
            nc.tensor.matmul(ps, w_sb, xT_sb[:, s], start=True, stop=True)
            nc.vector.tensor_mul(p_sb[:, s], ps, m_sb[:, s])

        def emit_proj_chunk(i):
            emit_proj_part(i, "q")
            emit_proj_part(i, "k")

        # v projection: 4 chunks share one PSUM slot and one strided DVE
        # copy into the 33-strided v_sb blocks.
        def emit_v_group(j4):
            ps_v = psum.tile([128, 4 * HD], F32, tag="s", name=f"ps_v_{j4}")
            for u in range(4):
                j = 4 * j4 + u
                nc.tensor.matmul(ps_v[:, u * HD:(u + 1) * HD],
                                 xT_sb[:, j * KC:(j + 1) * KC], wv_sb,
                                 start=True, stop=True, skip_group_check=True)
            v_view = v_sb[:, j4 * 4 * VW:(j4 * 4 + 4) * VW].rearrange(
                "p (j w) -> p j w", w=VW)[:, :, 0:HD]
            nc.vector.tensor_copy(
                v_view, ps_v[:, :].rearrange("p (j w) -> p j w", w=HD))

        emit_proj_chunk(0)
        emit_v_group(0)

        # ---- phase B: attention main loop ----
        # groups of 3 key-chunks share one 3-bank PSUM tile so exp runs as
        # one ACT instruction over [128, 1536]
        GROUPS = [3] * 10 + [2]
        if debug_taps:
            nc.gpsimd.dma_start(out=dbg["qp"][:, :], in_=qp_sb)
            nc.gpsimd.dma_start(out=dbg["kp"][:, :], in_=kp_sb)
            nc.gpsimd.dma_start(out=dbg["v"][:, :], in_=v_sb)
        # chunk k -> (group, slot) for the fixed GROUPS partition
        chunk_grp = []
        kb = 0
        for g, gs in enumerate(GROUPS):
            for t in range(gs):
                chunk_grp.append((g, t))
            kb += gs
        grp_base = []
        kb = 0
        for gs in GROUPS:
            grp_base.append(kb)
            kb += gs

        # Deferred per-qc tail (recip -> broadcast -> normalize -> final
        # projection -> DMA out). Emitted a few pairs INTO the next query
        # chunk so the in-order PE reaches the next chunk's score matmuls
        # (feeding ACT) before it queues behind the DVE normalize chain.
        def emit_norm_for(q, st):
            # No on-device softmax normalization: proj(out/Z) == proj(out)/Z
            # per token, so Z ships to the host (z_d) and the divide folds
            # into the host-side gather. Here: just cast outT to fp16.
            accs = st["accs"]
            outT = work.tile([HD, QC], BF16, tag="o", bufs=2,
                             name=f"outT_{q}")
            nc.vector.tensor_copy(outT, accs[0:HD, :])
            st["outT"] = outT
            st["os4"] = work.tile([128, 4, KEY_DIM], F32, tag="os", bufs=2,
                                  name=f"os4_{q}")

        def emit_proj_for(q, st, h2, dma_now=False):
            outT, os4 = st["outT"], st["os4"]
            pf = psum.tile([128, 2 * KEY_DIM], F32, tag="x", bufs=1,
                           name=f"pf_{q}_{h2}")
            for u in range(2):
                t = 2 * h2 + u
                nc.tensor.matmul(pf[:, u * KEY_DIM:(u + 1) * KEY_DIM],
                                 outT[:, t * 128:(t + 1) * 128], wf_sb,
                                 start=True, stop=True,
                                 skip_group_check=True)
            nc.vector.tensor_copy(os4[:, 2 * h2:2 * h2 + 2, :],
                                  pf[:, :].rearrange("p (u c) -> p u c",
                                                     c=KEY_DIM))
            if dma_now:
                out_view = out_d[:, :].rearrange(
                    "(qh t p) c -> qh p t c", t=2, p=128, qh=2 * NQC)
                nc.sync.dma_start(out=out_view[2 * q + h2],
                                  in_=os4[:, 2 * h2:2 * h2 + 2, :])

        def emit_out_dma_for(q, st):
            out_view = out_d[:, :].rearrange("(q t p) c -> q p t c", t=4, p=128)
            nc.sync.dma_start(out=out_view[q], in_=st["os4"])

        def emit_merge_for(q, acc, st):
            # Merge the two accumulators while copying out of PSUM (frees
            # the accumulator bank; the slow normalize chain runs on the
            # SBUF copy, deferred further into the next query chunk).
            accs = work.tile([VW, QC], F32, tag="accs", bufs=2,
                             name=f"accs_{q}")
            nc.vector.tensor_copy(accs, acc[0:VW, :])
            nc.vector.scalar_tensor_tensor(
                out=accs, in0=acc[64:64 + VW, :], scalar=1.0, in1=accs,
                op0=MULT, op1=ADD)
            if debug_taps and q == 0:
                nc.sync.dma_start(out=dbg["acc0"][:, :], in_=accs)
            nc.sync.dma_start(out=z_d[q:q + 1, :], in_=accs[HD:HD + 1, :])
            st["accs"] = accs

        # Work carried across the qc boundary, drained one item per score
        # pair so the in-order PE keeps feeding ACT with the next chunk's
        # scores instead of queueing behind the previous chunk's epilogue.
        deferred = []

        for q in range(NQC):
            qs = slice(q * QC, (q + 1) * QC)
            # Two accumulators (partitions 0-32 even chunks, 64-96 odd) so
            # attnv matmuls of a chunk pair run concurrently via disjoint
            # col-groups; merged after the loop.
            acc = psum.tile([97, QC], F32, tag="acc", bufs=1,
                            name=f"acc_{q}")

            # Scores are emitted in chunk PAIRS: even chunk uses array rows
            # 0-63, odd chunk rows 64-127 (duplicated q'/k' halves) -> the
            # two matmuls execute concurrently in the PE array. exp(g) is
            # emitted as soon as group g's last score lands; attnv(g) is
            # delayed by one group so the in-order PE never stalls on ACT.
            sc_tiles = {}
            emitted = [0] * len(GROUPS)
            attnv_q = []

            def emit_exp(g, q=q, sc_tiles=sc_tiles, attnv_q=attnv_q):
                gs = GROUPS[g]
                ex = work.tile([128, gs * QC], BF16, tag="e", bufs=3,
                               name=f"ex_{q}_{g}")
                nc.scalar.activation(ex, sc_tiles[g], EXP)
                if debug_taps and q == 0 and g == 0:
                    nc.gpsimd.dma_start(out=dbg["ex0"][:, :], in_=ex)
                attnv_q.append((g, ex))

            def emit_attnv(g, ex, acc=acc):
                gs = GROUPS[g]
                for t in range(gs):
                    k = grp_base[g] + t
                    odd = k % 2
                    nc.tensor.matmul(
                        acc[64:97, :] if odd else acc[0:33, :],
                        v_sb[:, k * VW:(k + 1) * VW],         # [128, 33]
                        ex[:, t * QC:(t + 1) * QC],           # [128, 512]
                        start=(k < 2), stop=(k >= NKC - 2),
                        tile_position=(0, 64) if odd else (0, 0),
                        skip_group_check=True)

            for p in range(NKC // 2):
                for k in (2 * p, 2 * p + 1):
                    g, slot = chunk_grp[k]
                    if g not in sc_tiles:
                        sc_tiles[g] = psum.tile(
                            [128, GROUPS[g] * QC], F32, tag="s", bufs=2,
                            name=f"sc_{q}_{g}")
                    half = 64 * (k % 2)
                    nc.tensor.matmul(
                        sc_tiles[g][:, slot * QC:(slot + 1) * QC],
                        kp_sb[half:half + 64, k * KC:(k + 1) * KC],
                        qp_sb[half:half + 64, qs],
                        start=True, stop=True)
                    emitted[g] += 1
                    if emitted[g] == GROUPS[g]:
                        emit_exp(g)
                        # lag attnv by one group behind exp
                        if len(attnv_q) >= 2:
                            emit_attnv(*attnv_q.pop(0))
                # previous-qc epilogue, one piece every other pair so each
                # item's producer is long done when the in-order PE gets
                # there (no mid-stream stalls in front of score matmuls)
                if deferred and p % 2 == 0:
                    deferred.pop(0)()
                if q == 0:
                    # rest of phase A, raced just ahead of its consumers;
                    # qp chunk i is only needed at qc i so those spread
                    # across later qcs (emitted one qc ahead)
                    if p % 2 == 0 and p // 2 + 1 < NQC:
                        emit_proj_part(p // 2 + 1, "k")
                    if p % 2 == 1 and (p + 1) // 2 < NKC // 4:
                        emit_v_group((p + 1) // 2)
                elif p == 9 and q + 1 < NQC:
                    emit_proj_part(q + 1, "q")
                if q == 0 and p == 15:
                    emit_proj_part(1, "q")

            # carry this qc's epilogue into the next qc's pair loop
            def flush(q=q, acc=acc, attnv_q=attnv_q, emit_attnv=emit_attnv):
                st = {}
                steps = []
                for g, ex in attnv_q:
                    steps.append(lambda g=g, ex=ex: emit_attnv(g, ex))
                steps.append(lambda: emit_merge_for(q, acc, st))
                steps.append(lambda: emit_norm_for(q, st))
                steps.append(lambda: emit_proj_for(q, st, 0))
                steps.append(lambda: emit_proj_for(q, st, 1))
                steps.append(lambda: emit_out_dma_for(q, st))
                return steps

            if q == NQC - 1:
                # final epilogue inline, DMA halves overlapped with proj
                st = {}
                for g, ex in attnv_q:
                    emit_attnv(g, ex)
                emit_merge_for(q, acc, st)
                emit_norm_for(q, st)
                emit_proj_for(q, st, 0, dma_now=True)
                emit_proj_for(q, st, 1, dma_now=True)
                deferred = []
            else:
                deferred = flush()

    nc.compile()
    return nc

    nc.compile()
    return nc


def _prepare_inputs(x, Wp, bp, Wf, bf):
    """Build per-core input maps (head h -> core h)."""
    x = np.ascontiguousarray(x, dtype=np.float32)
    Wp = np.ascontiguousarray(Wp, dtype=np.float32)
    bp = np.ascontiguousarray(bp, dtype=np.float32)
    Wf = np.ascontiguousarray(Wf, dtype=np.float32)
    bf = np.ascontiguousarray(bf, dtype=np.float32)

    r_w, theta = _polar_constants()
    inv_sqrt_hd = np.float32(1.0 / np.sqrt(np.float32(HD)))
    cos_t = np.cos(theta).astype(np.float32)
    sin_t = np.sin(theta).astype(np.float32)

    mcq = np.empty((128, N), dtype=np.float32)
    mcq[0:32, :] = cos_t * inv_sqrt_hd
    mcq[32:64, :] = sin_t * inv_sqrt_hd
    mcq[64:128, :] = mcq[0:64, :]
    mck = np.empty((128, N), dtype=np.float32)
    mck[0:32, :] = r_w * cos_t
    mck[32:64, :] = r_w * sin_t
    mck[64:128, :] = mck[0:64, :]

    xT = np.ascontiguousarray(x.reshape(N, C).T).astype(np.float16)  # [C, N]

    # NOTE: q/k biases (bp[0:512]) are NOT applied on device; they are zero
    # by the problem spec (fill=zeros). The v bias folds exactly into a
    # host-side output bias since softmax rows sum to 1:
    #   p @ (v + bv) @ Wf_h = p @ v @ Wf_h + bv @ Wf_h
    assert np.max(np.abs(bp[:2 * KEY_DIM])) == 0.0, "nonzero q/k bias unsupported"
    bv_full = bp[2 * KEY_DIM:3 * KEY_DIM]
    host_bias = (bf + bv_full @ Wf).astype(np.float32)  # [256]

    in_maps = []
    for h in range(NCORES):
        qs = slice(32 * h, 32 * h + 32)
        Wq = Wp[:, 0 * KEY_DIM:1 * KEY_DIM][:, qs]
        Wk = Wp[:, 1 * KEY_DIM:2 * KEY_DIM][:, qs]
        Wv = Wp[:, 2 * KEY_DIM:3 * KEY_DIM][:, qs]
        wqq = np.ascontiguousarray(
            np.concatenate([Wq, Wq, Wq, Wq], axis=1)).astype(np.float16)
        wkk = np.ascontiguousarray(
            np.concatenate([Wk, Wk, Wk, Wk], axis=1)).astype(np.float16)
        wf_h = np.ascontiguousarray(Wf[qs, :]).astype(np.float16)  # [32, 256]
        in_maps.append({
            "xT": xT, "mcq": mcq, "mck": mck,
            "wqq": wqq, "wkk": wkk,
            "wv": np.ascontiguousarray(Wv).astype(np.float16),
            "wf": wf_h,
        })
    return in_maps, host_bias


def kernel(x, Wp, bp, Wf, bf):
    from concourse.bass_utils import run_bass_kernel_spmd

    if "nc" not in _CACHE:
        _CACHE["nc"] = _build_nc()
    nc = _CACHE["nc"]

    in_maps, host_bias = _prepare_inputs(x, Wp, bp, Wf, bf)
    res = run_bass_kernel_spmd(nc, in_maps, core_ids=list(range(NCORES)))
    out = _combine_outputs(res.results)
    out = out + host_bias[None, :]
    return out.reshape(B, HI, WI, KEY_DIM).astype(np.float32)


def _combine_outputs(results):
    """Sum per-head partials, folding in the softmax denominators."""
    out = np.zeros((N, KEY_DIM), dtype=np.float32)
    for r in results:
        z = np.asarray(r["z"], dtype=np.float32).reshape(N, 1)
        out += np.asarray(r["out"], dtype=np.float32) / z
    return out
